# revision 43
# baseline (speedup 1.0000x reference)
"""Trainium2 Bass kernel for nn_AFFWithCustomGCN (SA-GC block + BN + residual relu).

Math (per batch n):
    Ah[h]   = A[h] * attn[n,h]                         # [H,V,V]
    feat    = einsum('ctv,hvw->hctw', x[n], Ah)        # aggregate over v
    pre     = einsum('hctw,hoc->otw', feat, Wd) + bd.sum(0)
    out     = relu(bn(pre) + x[n])                     # relu(relu(y)) == relu(y)

Data-parallel over batch N=256 across 8 cores (32/core), processed in quads
(4 batches) to fill 128 partitions.  x stays unpadded; chunks cover 5 whole
t's (125 (t,v) columns) so no v-padding waste: 25 full chunks + one 3-t tail
chunk (75 cols) per quad.

Per quad on device (phase-split so the PE stream is dense and HAM stays warm):
  Phase 1 (26 chunks j):
      pt[(t5 v)<=125, 96i+32h+o] = x_bf[chunk].T @ wd4
    (wd4 block-diag over batches zeroes cross-batch terms; x chunk is the
    stationary operand), then one PSUM->SBUF bf16 copy per chunk
    (DVE/ACT by parity) -> zt.
  Phase 2 (26 chunks): per head h=0..2, 4 batches in PE col groups:
      po[(i o), (t5 w)] += zt[:, 96i+32h:+32].T @ BD_h
    accumulated over heads in PSUM; accumulation group opened once per
    4-chunk bank window (start on first chunk, stop on last).
  Epilogue per window: DVE: tmp = po*scale + x (residual add),
  ACT: out = relu(tmp + shift).  BN params and summed conv bias folded
  into scale/shift.
"""

import numpy as np

import concourse.bass as bass
import concourse.tile as tile
from concourse import mybir
from concourse import bass2jax as _b2j
from concourse.bass_utils import run_bass_kernel_spmd


def _split_multi_waits(bir_json: bytes) -> bytes:
    """Walrus allows only one sync-wait per TPB instruction on several
    queue structs.  Split any instruction with >1 wait into preceding
    single-wait EventSemaphore instructions on the same engine (pure wait
    carriers, identical semantics)."""
    import orjson
    bir = orjson.loads(bir_json)
    ctr = 0
    for fn in bir.get("functions", []):
        for blk in fn.get("blocks", []):
            insts = blk.get("instructions")
            if not insts:
                continue
            out = []
            for inst in insts:
                si = inst.get("sync_info") or {}
                waits = si.get("on_wait") or []
                if len(waits) > 1:
                    eng = inst.get("engine")
                    for w in waits[:-1]:
                        out.append({
                            "debug": inst.get("debug", 0),
                            "engine": eng, "ins": [], "outs": [],
                            "name": f"WS-{ctr}",
                            "opcode": "EventSemaphore",
                            "sync_info": {"on_update": [], "on_wait": [w]},
                        })
                        ctr += 1
                    si["on_wait"] = [waits[-1]]
                out.append(inst)
            blk["instructions"] = out
    return orjson.dumps(bir)


_orig_compile_bir = _b2j.compile_bir_kernel


def _patched_compile_bir(bir_json, tmpdir, neff_name="file.neff"):
    return _orig_compile_bir(_split_multi_waits(bir_json), tmpdir,
                             neff_name=neff_name)


if _b2j.compile_bir_kernel is not _patched_compile_bir:
    _b2j.compile_bir_kernel = _patched_compile_bir

F32 = mybir.dt.float32
BF16 = mybir.dt.bfloat16

N, C, T, V, H = 256, 32, 128, 25, 3
TW = T * V                  # 3200
NCORES = 8
NSH = N // NCORES           # 32 batches per core
NQ = NSH // 4               # 8 quads per core
BN_EPS = 1e-5

# chunking: 5 whole t's per chunk (125 cols), 25 full + one 3-t tail
CHUNKS = [(125 * j, 125, 5 * j, 5) for j in range(25)] + [(3125, 75, 125, 3)]
# windows of 4 chunks sharing one PSUM bank (<=512 cols)
WINDOWS = [list(range(4 * w, 4 * w + 4)) for w in range(6)] + [[24, 25]]

_CACHE = {}


def _build_graph(nq=NQ):
    nc = bass.Bass()

    xp_d = nc.declare_dram_parameter("xp", [NSH, C, TW], BF16, isOutput=False)
    # v-major so each SBUF partition's diagonal-block DMA reads one
    # contiguous run
    ah_d = nc.declare_dram_parameter("ah", [V, NSH, 3, V], BF16, isOutput=False)
    wd4_d = nc.declare_dram_parameter("wd4", [128, 384], BF16, isOutput=False)
    scale_d = nc.declare_dram_parameter("scale", [128, 1], F32, isOutput=False)
    shift_d = nc.declare_dram_parameter("shift", [128, 1], F32, isOutput=False)
    out_d = nc.declare_dram_parameter("out", [NSH, C, TW], BF16, isOutput=True)

    with tile.TileContext(nc) as tc:
        with (
            tc.tile_pool(name="singles", bufs=1) as singles,
            tc.tile_pool(name="xpool", bufs=4) as xpool,
            tc.tile_pool(name="ztpool", bufs=28) as ztpool,
            tc.tile_pool(name="opool", bufs=2) as opool,
            tc.tile_pool(name="tpool", bufs=3) as tpool,
            tc.tile_pool(name="psPT", bufs=2, space="PSUM") as psPT,
            tc.tile_pool(name="psAcc", bufs=2, space="PSUM") as psAcc,
        ):
            wd4_sb = singles.tile([128, 384], BF16)
            nc.sync.dma_start(out=wd4_sb, in_=wd4_d[:, :])
            scale_sb = singles.tile([128, 1], F32)
            nc.sync.dma_start(out=scale_sb, in_=scale_d[:, :])
            shift_sb = singles.tile([128, 1], F32)
            nc.sync.dma_start(out=shift_sb, in_=shift_d[:, :])

            # Persistent block-diag Ah buffers (manual 3-deep rotation).
            # Zeros are set once; per quad only the 5 diagonal (t,t) blocks
            # are DMA'd in from the compact ah tensor -- the off-diagonal
            # zeros are never touched again.  Replaces the 3MB/core bdall
            # stream with 120KB/core.
            bd_bufs = []
            for bi in range(3):
                bb = singles.tile([125, 4, 375], BF16, name=f"bdbuf{bi}")
                if bi == 0:
                    nc.vector.memset(bb, 0.0)
                else:
                    nc.gpsimd.memset(bb, 0.0)
                bd_bufs.append(bb)

            # window ends -> (window idx) for epilogue triggering
            wend = {ch[-1]: w for w, ch in enumerate(WINDOWS)}
            wstart = {ch[0]: w for w, ch in enumerate(WINDOWS)}

            xs, bds, zts = {}, {}, {}

            def load_quad(q):
                # tiny adjacency DMAs first so they aren't queued behind
                # the 800KB x load in the rings
                bb = bd_bufs[q % 3]
                src = ah_d[:, 4 * q:4 * q + 4]
                # 5 diagonal-block DMAs (partition and column offsets both
                # scale with t -- inexpressible as one access pattern);
                # spread the issue cost across sequencers
                dma_eng = [nc.sync, nc.gpsimd, nc.sync, nc.gpsimd, nc.sync]
                for t in range(5):
                    dst = bb[25 * t:25 * t + 25].rearrange(
                        "p i (h w5) -> p i h w5", h=3)[:, :, :, 25 * t:25 * t + 25]
                    dma_eng[t].dma_start(out=dst, in_=src)
                bds[q] = bb
                xs[q] = xpool.tile([128, T, V], BF16, tag="x", name="xq")
                # halves so the first 64 t's worth of MM1 chunks can start
                # while the rest is in flight
                for (ta, tb) in ((0, 64), (64, 128)):
                    nc.gpsimd.dma_start(
                        out=xs[q][:, ta:tb, :],
                        in_=xp_d[4 * q:4 * q + 4].rearrange(
                            "n c (t v) -> (n c) t v", v=V)[:, ta:tb, :],
                    )

            # Warmup burst: dense junk matmuls fill the startup DMA wait
            # and flip HAM to K=8/8 before real work; with no >3.4us PE
            # idle mid-kernel it then stays warm.
            junk_ps = psPT.tile([128, 2, 512], F32, tag="pt", name="pt")
            for _ in range(20):
                nc.tensor.matmul(
                    junk_ps[:, 0, 0:384], wd4_sb[:, 0:128], wd4_sb,
                    start=True, stop=True)

            # Software-pipelined over quads: step s runs phase 1 (channel
            # mix + PSUM->SBUF copies) of quad s interleaved, window by
            # window, with phase 2 (propagation + epilogue) of quad s-1.
            load_quad(0)
            for s in range(nq + 1):
                q1 = s if s < nq else None       # quad in phase 1
                q2 = s - 1 if s >= 1 else None   # quad in phase 2
                if q1 is not None:
                    zts[q1] = []
                    if q1 + 1 < nq:
                        load_quad(q1 + 1)
                if q2 is not None:
                    out_sb = opool.tile([128, T, V], BF16)
                    x2 = xs[q2]
                    bd_q = bds[q2]

                # phase-1 chunks are emitted one at a time via this
                # closure so they can splice uniformly between phase-2
                # chunk groups (a fat 384-col stream every ~500ns keeps
                # the PE duty even for the HAM clock gate)
                p1_state = {"k": 0, "pt": None}

                def emit_p1_chunk():
                    k = p1_state["k"]
                    if q1 is None or k >= 26:
                        return
                    x_flat = xs[q1].rearrange("p t v -> p (t v)")
                    if k % 2 == 0:
                        p1_state["pt"] = psPT.tile(
                            [128, 2, 512], F32, tag="pt", name="pt")
                    pt = p1_state["pt"]
                    (c0, cw, tj, tn) = CHUNKS[k]
                    # pad weight loads to 128 cols (reads spill into the
                    # next chunk; rows cw:128 of pt are garbage, never
                    # consumed) so FWL (NumWeights==128) kicks in
                    cwp = 128 if c0 + 128 <= TW else cw
                    nc.tensor.matmul(
                        pt[0:cwp, k % 2, 0:384],
                        x_flat[:, c0:c0 + cwp],
                        wd4_sb,
                        start=True, stop=True,
                    )
                    if k % 2 == 1:
                        jp = k // 2
                        zt = ztpool.tile([128, 2, 384], BF16,
                                         tag="zt", name="zt")
                        eng = nc.vector.tensor_copy if jp % 2 == 0 \
                            else nc.scalar.copy
                        if k == 25:
                            # tail chunk only has 75 valid rows; copy
                            # slots separately to avoid reading
                            # unwritten PSUM rows
                            eng(zt[0:125, 0, :], pt[0:125, 0, 0:384])
                            eng(zt[0:75, 1, :], pt[0:75, 1, 0:384])
                        else:
                            eng(zt[0:125, :, :], pt[0:125, :, 0:384])
                        zts[q1].append(zt)
                    p1_state["k"] = k + 1

                for w, chunk_ids in enumerate(WINDOWS):
                    if q2 is not None:
                        # windows pair up in one 2-bank PSUM tile so the
                        # epilogue runs one 1000-col pass per pair
                        if w % 2 == 0:
                            po2 = psAcc.tile([128, 2, 512], F32,
                                             tag="po", name="po")
                        po = po2[:, w % 2, :]
                        off = 0
                        t0 = CHUNKS[chunk_ids[0]][2]
                        for jj, j in enumerate(chunk_ids):
                            (c0, cw, tj, tn) = CHUNKS[j]
                            zt = zts[q2][j // 2][:, j % 2, :]
                            for h in range(3):
                                for i in range(4):
                                    nc.tensor.matmul(
                                        po[32 * i:32 * (i + 1), off:off + cw],
                                        zt[0:cw, 96 * i + 32 * h:96 * i + 32 * h + 32],
                                        bd_q[0:cw, i, 125 * h:125 * h + cw],
                                        start=(h == 0 and jj == 0),
                                        stop=(h == 2 and jj == len(chunk_ids) - 1),
                                        tile_position=(0, 32 * i),
                                        # CoreSim's group-check mis-addresses
                                        # base-partition!=0 col-tiled outs;
                                        # the pattern is HW-validated.
                                        skip_group_check=True,
                                    )
                            off += cw

                        nt = off // V
                        if w % 2 == 0:
                            tmp2 = tpool.tile([128, 40, V], BF16,
                                              tag="tmp", name="tmp")
                        sl2 = 20 * (w % 2)
                        # tmp = po*scale + x  (residual add)
                        nc.vector.scalar_tensor_tensor(
                            tmp2[:, sl2:sl2 + nt, :],
                            po[:, 0:500].rearrange(
                                "p (t w5) -> p t w5", w5=V)[:, :nt, :],
                            scale_sb,
                            x2[:, t0:t0 + nt, :],
                            mybir.AluOpType.mult,
                            mybir.AluOpType.add,
                        )
                        # out = relu(tmp + shift) on ACT, one 2-window
                        # pass per pair (GpSimd's fixed overhead makes it
                        # ~6.7us/op there)
                        if w % 2 == 1 or w == len(WINDOWS) - 1:
                            ntp = sl2 + nt
                            t0p = 20 * (w - (1 if w % 2 == 1 else 0))
                            nc.scalar.activation(
                                out_sb[:, t0p:t0p + ntp, :],
                                tmp2[:, 0:ntp, :],
                                mybir.ActivationFunctionType.Relu,
                                bias=shift_sb, scale=1.0,
                            )

                    # phase-1 chunks for this window (after the window's
                    # phase-2 group so MM1 weight loads don't splice into
                    # the MM2 LDW stream mid-chain)
                    for _ in range(len(chunk_ids)):
                        emit_p1_chunk()

                if q2 is not None:
                    # output DMA in halves so the first half streams out
                    # while the second half is still being computed
                    for (ta, tb) in ((0, 64), (64, 128)):
                        nc.sync.dma_start(
                            out=out_d[4 * q2:4 * q2 + 4].rearrange(
                                "n c (t v) -> (n c) t v", v=V)[:, ta:tb, :],
                            in_=out_sb[:, ta:tb, :],
                        )
                    del zts[q2], xs[q2], bds[q2]

    return nc


def _prep_host(x, A, attn, Wd, bd, bn_gamma, bn_beta, bn_mean, bn_var):
    x = np.asarray(x, dtype=np.float32)
    A = np.asarray(A, dtype=np.float32)
    attn = np.asarray(attn, dtype=np.float32)
    Wd = np.asarray(Wd, dtype=np.float32)
    bd = np.asarray(bd, dtype=np.float32)
    bn_gamma = np.asarray(bn_gamma, dtype=np.float32)
    bn_beta = np.asarray(bn_beta, dtype=np.float32)
    bn_mean = np.asarray(bn_mean, dtype=np.float32)
    bn_var = np.asarray(bn_var, dtype=np.float32)

    xp = x.reshape(N, C, TW).astype(ml_bf16())

    scale = bn_gamma / np.sqrt(bn_var + BN_EPS)           # [32]
    shift = (bd.sum(axis=0) - bn_mean) * scale + bn_beta  # [32]

    # compact per-batch modulated adjacency; expanded to the 5-t block
    # diagonal on device.  v-major layout so each SBUF partition (= v)
    # reads one contiguous (n,h,w) run per quad.
    ah = (A[None, :, :, :] * attn).transpose(2, 0, 1, 3) \
        .astype(ml_bf16()).copy()                         # [V,N,H,V]

    # wd4: block-diag [ (i c), (4i x 3h x 32o) ]: wd4[32i+c, 96i+32h+o] = Wd[h,o,c]
    wdt = Wd.transpose(2, 0, 1).reshape(C, H * C)         # [c, (h o)]
    wd4 = np.zeros((128, 384), dtype=np.float32)
    for i in range(4):
        wd4[32 * i:32 * (i + 1), 96 * i:96 * (i + 1)] = wdt
    wd4 = wd4.astype(ml_bf16())                           # [128, 384]

    scale4 = np.tile(scale, 4).reshape(128, 1).astype(np.float32)
    shift4 = np.tile(shift, 4).reshape(128, 1).astype(np.float32)
    return xp, ah, wd4, scale4, shift4


def ml_bf16():
    import ml_dtypes
    return ml_dtypes.bfloat16


def kernel(x, A, attn, Wd, bd, bn_gamma, bn_beta, bn_mean, bn_var,
           _trace=False):
    xp, ah, wd4, scale4, shift4 = _prep_host(
        x, A, attn, Wd, bd, bn_gamma, bn_beta, bn_mean, bn_var)

    if "nc" not in _CACHE:
        _CACHE["nc"] = _build_graph()
    nc = _CACHE["nc"]

    in_maps = []
    for i in range(NCORES):
        sl = slice(i * NSH, (i + 1) * NSH)
        in_maps.append({
            "xp": xp[sl],
            "ah": np.ascontiguousarray(ah[:, sl]),
            "wd4": wd4,
            "scale": scale4,
            "shift": shift4,
        })

    res = run_bass_kernel_spmd(
        nc, in_maps, core_ids=list(range(NCORES)), trace=_trace,
    )
    out = np.concatenate([r["out"] for r in res.results], axis=0)
    out = out.astype(np.float32).reshape(N, C, T, V)
    if _trace:
        return out, res
    return out



# revision 46
# speedup vs baseline: 1.0815x; 1.0815x over previous
"""Trainium2 Bass kernel for nn_AFFWithCustomGCN (SA-GC block + BN + residual relu).

Math (per batch n):
    Ah[h]   = A[h] * attn[n,h]                         # [H,V,V]
    feat    = einsum('ctv,hvw->hctw', x[n], Ah)        # aggregate over v
    pre     = einsum('hctw,hoc->otw', feat, Wd) + bd.sum(0)
    out     = relu(bn(pre) + x[n])                     # relu(relu(y)) == relu(y)

Data-parallel over batch N=256 across 8 cores (32/core), processed in quads
(4 batches) to fill 128 partitions.  x stays unpadded; chunks cover 5 whole
t's (125 (t,v) columns) so no v-padding waste: 25 full chunks + one 3-t tail
chunk (75 cols) per quad.

Per quad on device (phase-split so the PE stream is dense and HAM stays warm):
  Phase 1 (26 chunks j):
      pt[(t5 v)<=125, 96i+32h+o] = x_bf[chunk].T @ wd4
    (wd4 block-diag over batches zeroes cross-batch terms; x chunk is the
    stationary operand), then one PSUM->SBUF bf16 copy per chunk
    (DVE/ACT by parity) -> zt.
  Phase 2 (26 chunks): per head h=0..2, 4 batches in PE col groups:
      po[(i o), (t5 w)] += zt[:, 96i+32h:+32].T @ BD_h
    accumulated over heads in PSUM; accumulation group opened once per
    4-chunk bank window (start on first chunk, stop on last).
  Epilogue per window: DVE: tmp = po*scale + x (residual add),
  ACT: out = relu(tmp + shift).  BN params and summed conv bias folded
  into scale/shift.
"""

import numpy as np

import concourse.bass as bass
import concourse.tile as tile
from concourse import mybir
from concourse import bass2jax as _b2j
from concourse.bass_utils import run_bass_kernel_spmd


def _split_multi_waits(bir_json: bytes) -> bytes:
    """Walrus allows only one sync-wait per TPB instruction on several
    queue structs.  Split any instruction with >1 wait into preceding
    single-wait EventSemaphore instructions on the same engine (pure wait
    carriers, identical semantics)."""
    import orjson
    bir = orjson.loads(bir_json)
    ctr = 0
    for fn in bir.get("functions", []):
        for blk in fn.get("blocks", []):
            insts = blk.get("instructions")
            if not insts:
                continue
            out = []
            for inst in insts:
                si = inst.get("sync_info") or {}
                waits = si.get("on_wait") or []
                if len(waits) > 1:
                    eng = inst.get("engine")
                    for w in waits[:-1]:
                        out.append({
                            "debug": inst.get("debug", 0),
                            "engine": eng, "ins": [], "outs": [],
                            "name": f"WS-{ctr}",
                            "opcode": "EventSemaphore",
                            "sync_info": {"on_update": [], "on_wait": [w]},
                        })
                        ctr += 1
                    si["on_wait"] = [waits[-1]]
                out.append(inst)
            blk["instructions"] = out
    return orjson.dumps(bir)


_orig_compile_bir = _b2j.compile_bir_kernel


def _patched_compile_bir(bir_json, tmpdir, neff_name="file.neff"):
    return _orig_compile_bir(_split_multi_waits(bir_json), tmpdir,
                             neff_name=neff_name)


if _b2j.compile_bir_kernel is not _patched_compile_bir:
    _b2j.compile_bir_kernel = _patched_compile_bir

F32 = mybir.dt.float32
BF16 = mybir.dt.bfloat16

N, C, T, V, H = 256, 32, 128, 25, 3
TW = T * V                  # 3200
NCORES = 8
NSH = N // NCORES           # 32 batches per core
NQ = NSH // 4               # 8 quads per core
BN_EPS = 1e-5

# chunking: 5 whole t's per chunk (125 cols), 25 full + one 3-t tail
CHUNKS = [(125 * j, 125, 5 * j, 5) for j in range(25)] + [(3125, 75, 125, 3)]
# windows of 4 chunks sharing one PSUM bank (<=512 cols)
WINDOWS = [list(range(4 * w, 4 * w + 4)) for w in range(6)] + [[24, 25]]

_CACHE = {}


def _build_graph(nq=NQ):
    nc = bass.Bass()

    xp_d = nc.declare_dram_parameter("xp", [NSH, C, TW], BF16, isOutput=False)
    # v-major so each SBUF partition's diagonal-block DMA reads one
    # contiguous run
    ah_d = nc.declare_dram_parameter("ah", [V, NSH, 3, V], BF16, isOutput=False)
    wd4_d = nc.declare_dram_parameter("wd4", [128, 384], BF16, isOutput=False)
    scale_d = nc.declare_dram_parameter("scale", [128, 1], F32, isOutput=False)
    shift_d = nc.declare_dram_parameter("shift", [128, 1], F32, isOutput=False)
    out_d = nc.declare_dram_parameter("out", [NSH, C, TW], BF16, isOutput=True)

    with tile.TileContext(nc) as tc:
        with (
            tc.tile_pool(name="singles", bufs=1) as singles,
            tc.tile_pool(name="xpool", bufs=4) as xpool,
            tc.tile_pool(name="ztpool", bufs=28) as ztpool,
            tc.tile_pool(name="opool", bufs=2) as opool,
            tc.tile_pool(name="tpool", bufs=3) as tpool,
            tc.tile_pool(name="psPT", bufs=2, space="PSUM") as psPT,
            tc.tile_pool(name="psAcc", bufs=4, space="PSUM") as psAcc,
        ):
            wd4_sb = singles.tile([128, 384], BF16)
            nc.sync.dma_start(out=wd4_sb, in_=wd4_d[:, :])
            scale_sb = singles.tile([128, 1], F32)
            nc.sync.dma_start(out=scale_sb, in_=scale_d[:, :])
            shift_sb = singles.tile([128, 1], F32)
            nc.sync.dma_start(out=shift_sb, in_=shift_d[:, :])

            # Persistent block-diag Ah buffers (manual 3-deep rotation).
            # Zeros are set once; per quad only the 5 diagonal (t,t) blocks
            # are DMA'd in from the compact ah tensor -- the off-diagonal
            # zeros are never touched again.  Replaces the 3MB/core bdall
            # stream with 120KB/core.
            bd_bufs = []
            for bi in range(3):
                bb = singles.tile([125, 4, 375], BF16, name=f"bdbuf{bi}")
                if bi == 0:
                    nc.vector.memset(bb, 0.0)
                else:
                    nc.gpsimd.memset(bb, 0.0)
                bd_bufs.append(bb)

            # window ends -> (window idx) for epilogue triggering
            wend = {ch[-1]: w for w, ch in enumerate(WINDOWS)}
            wstart = {ch[0]: w for w, ch in enumerate(WINDOWS)}

            xs, bds, zts = {}, {}, {}

            def load_quad(q):
                # tiny adjacency DMAs first so they aren't queued behind
                # the 800KB x load in the rings
                bb = bd_bufs[q % 3]
                src = ah_d[:, 4 * q:4 * q + 4]
                # 5 diagonal-block DMAs (partition and column offsets both
                # scale with t -- inexpressible as one access pattern);
                # spread the issue cost across sequencers
                dma_eng = [nc.sync, nc.gpsimd, nc.sync, nc.gpsimd, nc.sync]
                for t in range(5):
                    dst = bb[25 * t:25 * t + 25].rearrange(
                        "p i (h w5) -> p i h w5", h=3)[:, :, :, 25 * t:25 * t + 25]
                    dma_eng[t].dma_start(out=dst, in_=src)
                bds[q] = bb
                xs[q] = xpool.tile([128, T, V], BF16, tag="x", name="xq")
                # halves so the first 64 t's worth of MM1 chunks can start
                # while the rest is in flight
                for (ta, tb) in ((0, 64), (64, 128)):
                    nc.gpsimd.dma_start(
                        out=xs[q][:, ta:tb, :],
                        in_=xp_d[4 * q:4 * q + 4].rearrange(
                            "n c (t v) -> (n c) t v", v=V)[:, ta:tb, :],
                    )

            # Warmup burst: dense junk matmuls fill the startup DMA wait
            # and flip HAM to K=8/8 before real work; with no >3.4us PE
            # idle mid-kernel it then stays warm.
            junk_ps = psPT.tile([128, 2, 512], F32, tag="pt", name="pt")
            for _ in range(20):
                nc.tensor.matmul(
                    junk_ps[:, 0, 0:384], wd4_sb[:, 0:128], wd4_sb,
                    start=True, stop=True)

            # Software-pipelined over quads: step s runs phase 1 (channel
            # mix + PSUM->SBUF copies) of quad s interleaved, window by
            # window, with phase 2 (propagation + epilogue) of quad s-1.
            load_quad(0)
            for s in range(nq + 1):
                q1 = s if s < nq else None       # quad in phase 1
                q2 = s - 1 if s >= 1 else None   # quad in phase 2
                if q1 is not None:
                    zts[q1] = []
                    if q1 + 1 < nq:
                        load_quad(q1 + 1)
                if q2 is not None:
                    out_sb = opool.tile([128, T, V], BF16)
                    x2 = xs[q2]
                    bd_q = bds[q2]

                # phase-1 chunks are emitted one at a time via this
                # closure so they can splice uniformly between phase-2
                # chunk groups (a fat 384-col stream every ~500ns keeps
                # the PE duty even for the HAM clock gate)
                p1_state = {"k": 0, "pt": None}

                def emit_p1_chunk():
                    k = p1_state["k"]
                    if q1 is None or k >= 26:
                        return
                    x_flat = xs[q1].rearrange("p t v -> p (t v)")
                    if k % 2 == 0:
                        p1_state["pt"] = psPT.tile(
                            [128, 2, 512], F32, tag="pt", name="pt")
                    pt = p1_state["pt"]
                    (c0, cw, tj, tn) = CHUNKS[k]
                    # pad weight loads to 128 cols (reads spill into the
                    # next chunk; rows cw:128 of pt are garbage, never
                    # consumed) so FWL (NumWeights==128) kicks in
                    cwp = 128 if c0 + 128 <= TW else cw
                    nc.tensor.matmul(
                        pt[0:cwp, k % 2, 0:384],
                        x_flat[:, c0:c0 + cwp],
                        wd4_sb,
                        start=True, stop=True,
                    )
                    if k % 2 == 1:
                        jp = k // 2
                        zt = ztpool.tile([128, 2, 384], BF16,
                                         tag="zt", name="zt")
                        eng = nc.vector.tensor_copy if jp % 2 == 0 \
                            else nc.scalar.copy
                        if k == 25:
                            # tail chunk only has 75 valid rows; copy
                            # slots separately to avoid reading
                            # unwritten PSUM rows
                            eng(zt[0:125, 0, :], pt[0:125, 0, 0:384])
                            eng(zt[0:75, 1, :], pt[0:75, 1, 0:384])
                        else:
                            eng(zt[0:125, :, :], pt[0:125, :, 0:384])
                        zts[q1].append(zt)
                    p1_state["k"] = k + 1

                for w, chunk_ids in enumerate(WINDOWS):
                    if q2 is not None:
                        po = psAcc.tile([128, 512], F32, tag="po", name="po")
                        off = 0
                        t0 = CHUNKS[chunk_ids[0]][2]
                        for jj, j in enumerate(chunk_ids):
                            (c0, cw, tj, tn) = CHUNKS[j]
                            zt = zts[q2][j // 2][:, j % 2, :]
                            for h in range(3):
                                for i in range(4):
                                    nc.tensor.matmul(
                                        po[32 * i:32 * (i + 1), off:off + cw],
                                        zt[0:cw, 96 * i + 32 * h:96 * i + 32 * h + 32],
                                        bd_q[0:cw, i, 125 * h:125 * h + cw],
                                        start=(h == 0 and jj == 0),
                                        stop=(h == 2 and jj == len(chunk_ids) - 1),
                                        tile_position=(0, 32 * i),
                                        # CoreSim's group-check mis-addresses
                                        # base-partition!=0 col-tiled outs;
                                        # the pattern is HW-validated.
                                        skip_group_check=True,
                                    )
                            off += cw

                        nt = off // V
                        tmp = tpool.tile([128, 20, V], BF16,
                                         tag="tmp", name="tmp")
                        # tmp = po*scale + x  (residual add)
                        nc.vector.scalar_tensor_tensor(
                            tmp[:, :nt, :],
                            po[:, 0:500].rearrange(
                                "p (t w5) -> p t w5", w5=V)[:, :nt, :],
                            scale_sb,
                            x2[:, t0:t0 + nt, :],
                            mybir.AluOpType.mult,
                            mybir.AluOpType.add,
                        )
                        # out = relu(tmp + shift) on ACT (GpSimd's fixed
                        # overhead makes it ~6.7us/op -- 16x slower)
                        nc.scalar.activation(
                            out_sb[:, t0:t0 + nt, :], tmp[:, :nt, :],
                            mybir.ActivationFunctionType.Relu,
                            bias=shift_sb, scale=1.0,
                        )

                    # phase-1 chunks for this window (after the window's
                    # phase-2 group so MM1 weight loads don't splice into
                    # the MM2 LDW stream mid-chain)
                    for _ in range(len(chunk_ids)):
                        emit_p1_chunk()

                if q2 is not None:
                    # output DMA in halves so the first half streams out
                    # while the second half is still being computed
                    for (ta, tb) in ((0, 64), (64, 128)):
                        nc.sync.dma_start(
                            out=out_d[4 * q2:4 * q2 + 4].rearrange(
                                "n c (t v) -> (n c) t v", v=V)[:, ta:tb, :],
                            in_=out_sb[:, ta:tb, :],
                        )
                    del zts[q2], xs[q2], bds[q2]

    return nc


def _prep_host(x, A, attn, Wd, bd, bn_gamma, bn_beta, bn_mean, bn_var):
    x = np.asarray(x, dtype=np.float32)
    A = np.asarray(A, dtype=np.float32)
    attn = np.asarray(attn, dtype=np.float32)
    Wd = np.asarray(Wd, dtype=np.float32)
    bd = np.asarray(bd, dtype=np.float32)
    bn_gamma = np.asarray(bn_gamma, dtype=np.float32)
    bn_beta = np.asarray(bn_beta, dtype=np.float32)
    bn_mean = np.asarray(bn_mean, dtype=np.float32)
    bn_var = np.asarray(bn_var, dtype=np.float32)

    xp = x.reshape(N, C, TW).astype(ml_bf16())

    scale = bn_gamma / np.sqrt(bn_var + BN_EPS)           # [32]
    shift = (bd.sum(axis=0) - bn_mean) * scale + bn_beta  # [32]

    # compact per-batch modulated adjacency; expanded to the 5-t block
    # diagonal on device.  v-major layout so each SBUF partition (= v)
    # reads one contiguous (n,h,w) run per quad.
    ah = (A[None, :, :, :] * attn).transpose(2, 0, 1, 3) \
        .astype(ml_bf16()).copy()                         # [V,N,H,V]

    # wd4: block-diag [ (i c), (4i x 3h x 32o) ]: wd4[32i+c, 96i+32h+o] = Wd[h,o,c]
    wdt = Wd.transpose(2, 0, 1).reshape(C, H * C)         # [c, (h o)]
    wd4 = np.zeros((128, 384), dtype=np.float32)
    for i in range(4):
        wd4[32 * i:32 * (i + 1), 96 * i:96 * (i + 1)] = wdt
    wd4 = wd4.astype(ml_bf16())                           # [128, 384]

    scale4 = np.tile(scale, 4).reshape(128, 1).astype(np.float32)
    shift4 = np.tile(shift, 4).reshape(128, 1).astype(np.float32)
    return xp, ah, wd4, scale4, shift4


def ml_bf16():
    import ml_dtypes
    return ml_dtypes.bfloat16


def kernel(x, A, attn, Wd, bd, bn_gamma, bn_beta, bn_mean, bn_var,
           _trace=False):
    xp, ah, wd4, scale4, shift4 = _prep_host(
        x, A, attn, Wd, bd, bn_gamma, bn_beta, bn_mean, bn_var)

    if "nc" not in _CACHE:
        _CACHE["nc"] = _build_graph()
    nc = _CACHE["nc"]

    in_maps = []
    for i in range(NCORES):
        sl = slice(i * NSH, (i + 1) * NSH)
        in_maps.append({
            "xp": xp[sl],
            "ah": np.ascontiguousarray(ah[:, sl]),
            "wd4": wd4,
            "scale": scale4,
            "shift": shift4,
        })

    res = run_bass_kernel_spmd(
        nc, in_maps, core_ids=list(range(NCORES)), trace=_trace,
    )
    out = np.concatenate([r["out"] for r in res.results], axis=0)
    out = out.astype(np.float32).reshape(N, C, T, V)
    if _trace:
        return out, res
    return out

